# revision 4
# baseline (speedup 1.0000x reference)
"""Grouped-experts SwiGLU MoE kernel for Trainium2 (8 NeuronCores).

Expert-parallel sharding: core e owns expert e's weights and its contiguous
token group (m_sizes gives T//E = 2048 tokens per expert). No collectives —
routing/scatter/gather happens on the host, each core runs an identical
single-core program on its own shard.

Per-core math: out = (silu(x_e @ w1_e) * (x_e @ w3_e)) @ w2_e
  x_e [2048, 2048], w1/w3 [2048, 1024], w2 [1024, 2048].

Device strategy (all-bf16 datapath, fp32 PSUM accumulation):
  phase 1 (up+gate):  stationary = w1/w3 128x128 tiles (bf16 -> FWL loads),
      moving = xT tiles (bf16, pre-transposed on host so D is the
      partition/contraction axis). PSUM accumulates over D; SwiGLU evac
      (ACT silu + DVE mul) writes the intermediate zT [H, M] as bf16.
  phase 2 (down):     stationary = zT 128x128 tiles (bf16), moving = w2
      tiles (bf16, resident in SBUF). PSUM accumulates over H; DVE copies
      to SBUF as bf16 and DMA stores out [M, D]; host upcasts to f32.

DMA plan: two HWDGE rings. The sync ring carries the x stream (both token
halves issued up front) and the output writes; the scalar ring carries the
weight streams. w1/w3 for iteration h+1 are triggered at the top of
iteration h (before h's silu in ACT program order) so the prefetch is never
queued behind an activation's semaphore wait; w2 h-chunks are spread across
phase 1 of half 0. bf16 everywhere halves HBM traffic vs f32/f32r and makes
LDWEIGHTS eligible for fast-weight-load, so the PE stays within a few
percent of its 1 cycle/row roofline.

Tokens are processed in two halves of 1024 so u/g PSUM accumulators
(2+2 banks) double-buffer across h iterations within the 8 PSUM banks.
"""

import numpy as np
import ml_dtypes

E, T, D, H = 8, 16384, 2048, 1024
M = T // E            # tokens per expert
P = 128
DC = D // P           # 16 contraction chunks (phase 1)
HC = H // P           # 8 contraction chunks (phase 2)
NHALF = 2
MH = M // NHALF       # 1024 tokens per half
NMOV = 512            # moving free dim / PSUM bank width (f32)
WSPL = 2              # w1/w3 per-h loads split into WSPL DMAs

_CACHE = {}
LAST_RESULTS = None   # for test harnesses that want the profile


def _build_program():
    import concourse.bacc as bacc
    import concourse.bass as bass
    import concourse.mybir as mybir
    import concourse.tile as tile

    f32 = mybir.dt.float32
    bf16 = mybir.dt.bfloat16
    SILU = mybir.ActivationFunctionType.Silu

    nc = bacc.Bacc("TRN2", target_bir_lowering=False, debug=False)

    xT = nc.dram_tensor("xT", [DC, P, M], bf16, kind="ExternalInput")
    w1r = nc.dram_tensor("w1r", [HC, P, DC, P], bf16, kind="ExternalInput")
    w3r = nc.dram_tensor("w3r", [HC, P, DC, P], bf16, kind="ExternalInput")
    w2r = nc.dram_tensor("w2r", [HC, P, D], bf16, kind="ExternalInput")
    out = nc.dram_tensor("out", [M, D], bf16, kind="ExternalOutput")

    with tile.TileContext(nc) as tc:
        with (
            tc.tile_pool(name="xp", bufs=1) as xp,
            tc.tile_pool(name="w2p", bufs=1) as w2p,
            tc.tile_pool(name="zp", bufs=1) as zp,
            tc.tile_pool(name="wp", bufs=2) as wp,
            tc.tile_pool(name="op", bufs=2) as op,
            tc.tile_pool(name="sp", bufs=3) as sp,
            tc.tile_pool(name="ps", bufs=2, space=bass.MemorySpace.PSUM) as ps,
        ):
            w2t = w2p.tile([P, HC, D], bf16, tag="w2", name="w2t")
            xts = [xp.tile([P, DC, MH], bf16, tag=f"xt{hf}", name=f"xt{hf}") for hf in range(NHALF)]
            zts = [zp.tile([P, HC, MH], bf16, tag=f"zt{hf}", name=f"zt{hf}") for hf in range(NHALF)]

            # HAM warm-up: the PE clock-gate defaults to 1.2 GHz and takes
            # ~3.4us of sustained activity to release. Run dummy matmuls on a
            # zeroed tile during the initial DMA wait so the real matmuls
            # start at 2.4 GHz.
            wtile = sp.tile([P, P + NMOV], bf16, tag="warm", name="wtile")
            nc.vector.memset(wtile[:], 0.0)
            pwarm = ps.tile([P, NMOV], f32, tag="p0", name="pwarm")
            for _ in range(30):
                nc.tensor.matmul(
                    pwarm[:], wtile[:, 0:P], wtile[:, P:P + NMOV],
                    start=True, stop=True,
                )

            # x stream: both halves issued up front on the sync ring; the
            # very first chunk is split so the first matmul only waits for
            # 128 KB.
            for hf in range(NHALF):
                msl = slice(hf * MH, (hf + 1) * MH)
                for c in range(DC):
                    if hf == 0 and c == 0:
                        nc.sync.dma_start(xts[0][:, 0, 0:NMOV], xT[0, :, 0:NMOV])
                        nc.sync.dma_start(xts[0][:, 0, NMOV:MH], xT[0, :, NMOV:MH])
                    else:
                        nc.sync.dma_start(xts[hf][:, c, :], xT[c, :, msl])

            first_w = [True]

            def load_w13(h):
                # one h-iteration of w1+w3 (1 MB), split so the first matmul
                # of an iteration only depends on a small slice
                w1t = wp.tile([P, DC, P], bf16, tag="w1", name=f"w1t{h}")
                w3t = wp.tile([P, DC, P], bf16, tag="w3", name=f"w3t{h}")
                if first_w[0]:
                    # chunk 0 alone (32 KB) so the first matmul starts asap
                    bounds = [0, 1, DC // 2, DC]
                    first_w[0] = False
                else:
                    bounds = [s * DC // WSPL for s in range(WSPL + 1)]
                for lo, hi in zip(bounds, bounds[1:]):
                    csl = slice(lo, hi)
                    nc.scalar.dma_start(w1t[:, csl, :], w1r[h, :, csl, :])
                    nc.scalar.dma_start(w3t[:, csl, :], w3r[h, :, csl, :])
                return w1t, w3t

            for hf in range(NHALF):
                xt = xts[hf]
                zt = zts[hf]

                # ---- phase 1: u = x@w1, g = x@w3, z = silu(u)*g ----
                if hf == 0:
                    wcur = load_w13(0)
                for h in range(HC):
                    # prefetch next iteration's weights BEFORE this
                    # iteration's silu enters the ACT queue
                    if h + 1 < HC:
                        wnxt = load_w13(h + 1)
                    if hf == 0 and h >= 1:
                        # w2 h-chunks ride the scalar ring during phase 1,
                        # deferred one iteration to keep h=0's HBM bandwidth
                        # free for the x stream
                        nc.scalar.dma_start(w2t[:, h - 1, :], w2r[h - 1])
                    w1t, w3t = wcur
                    pu = [ps.tile([P, NMOV], f32, tag=f"p{i}", name=f"pu{i}") for i in range(2)]
                    pg = [ps.tile([P, NMOV], f32, tag=f"p{i + 2}", name=f"pg{i}") for i in range(2)]
                    for c in range(DC):
                        first, last = c == 0, c == DC - 1
                        for mi in range(MH // NMOV):
                            nc.tensor.matmul(
                                pu[mi][:], w1t[:, c, :],
                                xt[:, c, mi * NMOV:(mi + 1) * NMOV],
                                start=first, stop=last,
                            )
                        for mi in range(MH // NMOV):
                            nc.tensor.matmul(
                                pg[mi][:], w3t[:, c, :],
                                xt[:, c, mi * NMOV:(mi + 1) * NMOV],
                                start=first, stop=last,
                            )
                    if h + 1 < HC:
                        wcur = wnxt
                    elif hf == 0:
                        # prefetch half-1 h=0 weights; fires at the start
                        # of half-0 phase 2, needed ~55us later
                        wcur = load_w13(0)
                        nc.scalar.dma_start(w2t[:, HC - 2, :], w2r[HC - 2])
                        nc.scalar.dma_start(w2t[:, HC - 1, :], w2r[HC - 1])
                    for mi in range(MH // NMOV):
                        st = sp.tile([P, NMOV], f32, tag="st", name="st")
                        nc.scalar.activation(st[:], pu[mi][:], SILU)
                        nc.vector.tensor_mul(
                            zt[:, h, mi * NMOV:(mi + 1) * NMOV],
                            st[:], pg[mi][:],
                        )

                # ---- phase 2: out = z @ w2 ----
                for mi in range(MH // P):
                    po = [ps.tile([P, NMOV], f32, tag=f"p{dd}", name=f"po{dd}") for dd in range(4)]
                    for h in range(HC):
                        zst = zt[:, h, mi * P:(mi + 1) * P]
                        for dd in range(D // NMOV):
                            nc.tensor.matmul(
                                po[dd][:], zst,
                                w2t[:, h, dd * NMOV:(dd + 1) * NMOV],
                                start=h == 0, stop=h == HC - 1,
                            )
                    osb = op.tile([P, D], bf16, tag="o", name="osb")
                    for dd in range(D // NMOV):
                        # split the PSUM evac across DVE and ACT so the last
                        # row block drains in half the time
                        dst = osb[:, dd * NMOV:(dd + 1) * NMOV]
                        if dd % 2 == 0:
                            nc.vector.tensor_copy(dst, po[dd][:])
                        else:
                            nc.scalar.activation(
                                dst, po[dd][:],
                                mybir.ActivationFunctionType.Copy,
                            )
                    r0 = hf * MH + mi * P
                    # split write: first half fires once dd=0,1 are copied
                    hd = D // 2
                    nc.sync.dma_start(out[r0:r0 + P, 0:hd], osb[:, 0:hd])
                    nc.sync.dma_start(out[r0:r0 + P, hd:D], osb[:, hd:D])

    nc.compile()
    return nc


def _get_program():
    if "nc" not in _CACHE:
        _CACHE["nc"] = _build_program()
    return _CACHE["nc"]


def _prep_w13(w):
    # [D, H] -> [HC, P, DC, P]; element [h,p,c,m] = w[c*P+p, h*P+m]
    bf = w.astype(ml_dtypes.bfloat16)
    return np.ascontiguousarray(
        bf.reshape(DC, P, HC, P).transpose(2, 1, 0, 3)
    )


def _numpy_fallback(x, w1, w2, w3, m_sizes):
    offs = np.concatenate([[0], np.cumsum(np.asarray(m_sizes, dtype=np.int64))])
    out = np.zeros((x.shape[0], w2.shape[2]), dtype=np.float32)
    for e in range(w1.shape[0]):
        xe = x[offs[e]:offs[e + 1]]
        u = xe @ w1[e]
        g = xe @ w3[e]
        z = (u / (1.0 + np.exp(-u))) * g
        out[offs[e]:offs[e + 1]] = z @ w2[e]
    return out


def kernel(x, w1, w2, w3, m_sizes, _trace=False, _trace_kwargs=None):
    global LAST_RESULTS
    x = np.ascontiguousarray(x, dtype=np.float32)
    w1 = np.ascontiguousarray(w1, dtype=np.float32)
    w2 = np.ascontiguousarray(w2, dtype=np.float32)
    w3 = np.ascontiguousarray(w3, dtype=np.float32)
    m = np.asarray(m_sizes, dtype=np.int64)

    expected = (
        x.shape == (T, D)
        and w1.shape == (E, D, H)
        and w2.shape == (E, H, D)
        and w3.shape == (E, D, H)
        and m.shape == (E,)
        and np.all(m == M)
    )
    if not expected:
        return _numpy_fallback(x, w1, w2, w3, m_sizes)

    from concourse.bass_utils import run_bass_kernel_spmd

    nc = _get_program()
    bf = ml_dtypes.bfloat16
    in_maps = []
    for e in range(E):
        xe = x[e * M:(e + 1) * M].astype(bf)
        in_maps.append({
            # [DC, P, M]; element [c,p,m] = x_e[m, c*P+p]
            "xT": np.ascontiguousarray(xe.T.reshape(DC, P, M)),
            "w1r": _prep_w13(w1[e]),
            "w3r": _prep_w13(w3[e]),
            "w2r": np.ascontiguousarray(w2[e].astype(bf).reshape(HC, P, D)),
        })

    res = run_bass_kernel_spmd(
        nc, in_maps, core_ids=list(range(E)),
        trace=_trace, **(_trace_kwargs or {}),
    )
    LAST_RESULTS = res
    return np.concatenate(
        [r["out"].astype(np.float32) for r in res.results], axis=0
    )


# revision 5
# speedup vs baseline: 1.0279x; 1.0279x over previous
"""Grouped-experts SwiGLU MoE kernel for Trainium2 (8 NeuronCores).

Expert-parallel sharding: core e owns expert e's weights and its contiguous
token group (m_sizes gives T//E = 2048 tokens per expert). No collectives —
routing/scatter/gather happens on the host, each core runs an identical
single-core program on its own shard.

Per-core math: out = (silu(x_e @ w1_e) * (x_e @ w3_e)) @ w2_e
  x_e [2048, 2048], w1/w3 [2048, 1024], w2 [1024, 2048].

Device strategy (all-bf16 datapath, fp32 PSUM accumulation):
  phase 1 (up+gate):  stationary = w1/w3 128x128 tiles (bf16), moving = xT
      tiles (bf16, pre-transposed on host so D is the partition/contraction
      axis). PSUM accumulates over D; SwiGLU evac (ACT silu + DVE mul)
      writes the intermediate zT [H, M] as bf16.
  phase 2 (down):     stationary = zT 128x128 tiles (bf16), moving = w2
      tiles (bf16, resident in SBUF). PSUM accumulates over H; evac splits
      across DVE/ACT, casts to bf16, DMA stores out [M, D]; host upcasts.

Memory/DMA plan:
  - w1/w3 are loaded ONCE into SBUF (8 MB resident) on the scalar HWDGE
    ring, paced one h-iteration ahead of the matmuls that need them, so
    half 1's phase 1 runs with zero weight traffic.
  - x streams on the sync HWDGE ring: half 0 up front (the first matmul
    waits only for a 128 KB slice), half 1 issued at the start of half-0
    phase 2 into the same buffer (WAR-safe: phase 1 of half 0 is done).
  - w2 h-chunks ride the scalar ring during phase-1 iterations, deferred
    one iteration so h=0 keeps the full HBM bandwidth for x.
  - Output is written bf16 (halves write traffic; host upcasts to f32).
  - ~30 dummy matmuls on a zeroed tile run during the initial DMA wait to
    release the PE HAM clock-gate (cold 1.2 GHz -> warm 2.4 GHz) before
    the real matmuls start.

Tokens are processed in two halves of 1024 so u/g PSUM accumulators
(2+2 banks) double-buffer across h iterations within the 8 PSUM banks.
"""

import numpy as np
import ml_dtypes

E, T, D, H = 8, 16384, 2048, 1024
M = T // E            # tokens per expert
P = 128
DC = D // P           # 16 contraction chunks (phase 1)
HC = H // P           # 8 contraction chunks (phase 2)
NHALF = 2
MH = M // NHALF       # 1024 tokens per half
NMOV = 512            # moving free dim / PSUM bank width (f32)
WSPL = 2              # w1/w3 per-h loads split into WSPL DMAs

_CACHE = {}
LAST_RESULTS = None   # for test harnesses that want the profile


def _build_program():
    import concourse.bacc as bacc
    import concourse.bass as bass
    import concourse.mybir as mybir
    import concourse.tile as tile

    f32 = mybir.dt.float32
    bf16 = mybir.dt.bfloat16
    SILU = mybir.ActivationFunctionType.Silu
    COPY = mybir.ActivationFunctionType.Copy

    nc = bacc.Bacc("TRN2", target_bir_lowering=False, debug=False)

    xT = nc.dram_tensor("xT", [DC, P, M], bf16, kind="ExternalInput")
    w1r = nc.dram_tensor("w1r", [HC, P, DC, P], bf16, kind="ExternalInput")
    w3r = nc.dram_tensor("w3r", [HC, P, DC, P], bf16, kind="ExternalInput")
    w2r = nc.dram_tensor("w2r", [HC, P, D], bf16, kind="ExternalInput")
    out = nc.dram_tensor("out", [M, D], bf16, kind="ExternalOutput")

    with tile.TileContext(nc) as tc:
        with (
            tc.tile_pool(name="xp", bufs=1) as xp,
            tc.tile_pool(name="wp", bufs=1) as wp,
            tc.tile_pool(name="zp", bufs=1) as zp,
            tc.tile_pool(name="op", bufs=2) as op,
            tc.tile_pool(name="sp", bufs=3) as sp,
            tc.tile_pool(name="ps", bufs=2, space=bass.MemorySpace.PSUM) as ps,
        ):
            w2t = wp.tile([P, HC, D], bf16, tag="w2", name="w2t")
            w1R = wp.tile([P, HC, DC, P], bf16, tag="w1R", name="w1R")
            w3R = wp.tile([P, HC, DC, P], bf16, tag="w3R", name="w3R")
            xt = xp.tile([P, DC, MH], bf16, tag="xt", name="xt")
            zts = [zp.tile([P, HC, MH], bf16, tag=f"zt{hf}", name=f"zt{hf}") for hf in range(NHALF)]

            # HAM warm-up: the PE clock-gate defaults to 1.2 GHz and takes
            # ~3.4us of sustained activity to release. Run dummy matmuls on
            # a zeroed tile during the initial DMA wait so the real matmuls
            # start at 2.4 GHz.
            wtile = sp.tile([P, P + NMOV], bf16, tag="warm", name="wtile")
            nc.vector.memset(wtile[:], 0.0)
            pwarm = ps.tile([P, NMOV], f32, tag="p0", name="pwarm")
            for _ in range(30):
                nc.tensor.matmul(
                    pwarm[:], wtile[:, 0:P], wtile[:, P:P + NMOV],
                    start=True, stop=True,
                )

            # half-0 x stream on the sync ring; the very first chunk is
            # split so the first matmul only waits for 128 KB
            nc.sync.dma_start(xt[:, 0, 0:NMOV], xT[0, :, 0:NMOV])
            nc.sync.dma_start(xt[:, 0, NMOV:MH], xT[0, :, NMOV:MH])
            for c in range(1, DC):
                nc.sync.dma_start(xt[:, c, :], xT[c, :, 0:MH])

            first_w = [True]

            def load_w13(h):
                # one h-iteration of w1+w3 (1 MB) into the resident buffers,
                # split so the first matmul only depends on a small slice
                if first_w[0]:
                    bounds = [0, 1, DC // 2, DC]
                    first_w[0] = False
                else:
                    bounds = [s * DC // WSPL for s in range(WSPL + 1)]
                for lo, hi in zip(bounds, bounds[1:]):
                    csl = slice(lo, hi)
                    nc.scalar.dma_start(w1R[:, h, csl, :], w1r[h, :, csl, :])
                    nc.scalar.dma_start(w3R[:, h, csl, :], w3r[h, :, csl, :])

            for hf in range(NHALF):
                zt = zts[hf]

                # ---- phase 1: u = x@w1, g = x@w3, z = silu(u)*g ----
                if hf == 0:
                    load_w13(0)
                for h in range(HC):
                    if hf == 0:
                        # prefetch next iteration's weights BEFORE this
                        # iteration's silu enters the ACT queue
                        if h + 1 < HC:
                            load_w13(h + 1)
                        if h >= 1:
                            # w2 rides the scalar ring, deferred one
                            # iteration to keep h=0's bandwidth for x
                            nc.scalar.dma_start(w2t[:, h - 1, :], w2r[h - 1])
                    pu = [ps.tile([P, NMOV], f32, tag=f"p{i}", name=f"pu{i}") for i in range(2)]
                    pg = [ps.tile([P, NMOV], f32, tag=f"p{i + 2}", name=f"pg{i}") for i in range(2)]
                    for c in range(DC):
                        first, last = c == 0, c == DC - 1
                        for mi in range(MH // NMOV):
                            nc.tensor.matmul(
                                pu[mi][:], w1R[:, h, c, :],
                                xt[:, c, mi * NMOV:(mi + 1) * NMOV],
                                start=first, stop=last,
                            )
                        for mi in range(MH // NMOV):
                            nc.tensor.matmul(
                                pg[mi][:], w3R[:, h, c, :],
                                xt[:, c, mi * NMOV:(mi + 1) * NMOV],
                                start=first, stop=last,
                            )
                    if hf == 0 and h == HC - 1:
                        nc.scalar.dma_start(w2t[:, HC - 2, :], w2r[HC - 2])
                        nc.scalar.dma_start(w2t[:, HC - 1, :], w2r[HC - 1])
                    for mi in range(MH // NMOV):
                        st = sp.tile([P, NMOV], f32, tag="st", name="st")
                        nc.scalar.activation(st[:], pu[mi][:], SILU)
                        nc.vector.tensor_mul(
                            zt[:, h, mi * NMOV:(mi + 1) * NMOV],
                            st[:], pg[mi][:],
                        )

                if hf == 0:
                    # half-1 x stream into the same buffer; WAR-safe (half-0
                    # phase 1 is done) and fully covered by half-0 phase 2
                    for c in range(DC):
                        nc.sync.dma_start(xt[:, c, :], xT[c, :, MH:M])

                # ---- phase 2: out = z @ w2 ----
                for mi in range(MH // P):
                    po = [ps.tile([P, NMOV], f32, tag=f"p{dd}", name=f"po{dd}") for dd in range(4)]
                    for h in range(HC):
                        zst = zt[:, h, mi * P:(mi + 1) * P]
                        for dd in range(D // NMOV):
                            nc.tensor.matmul(
                                po[dd][:], zst,
                                w2t[:, h, dd * NMOV:(dd + 1) * NMOV],
                                start=h == 0, stop=h == HC - 1,
                            )
                    osb = op.tile([P, D], bf16, tag="o", name="osb")
                    for dd in range(D // NMOV):
                        # split the PSUM evac across DVE and ACT so the last
                        # row block drains in half the time
                        dst = osb[:, dd * NMOV:(dd + 1) * NMOV]
                        if dd % 2 == 0:
                            nc.vector.tensor_copy(dst, po[dd][:])
                        else:
                            nc.scalar.activation(dst, po[dd][:], COPY)
                    r0 = hf * MH + mi * P
                    hd = D // 2
                    nc.sync.dma_start(out[r0:r0 + P, 0:hd], osb[:, 0:hd])
                    nc.sync.dma_start(out[r0:r0 + P, hd:D], osb[:, hd:D])

    nc.compile()
    return nc


def _get_program():
    if "nc" not in _CACHE:
        _CACHE["nc"] = _build_program()
    return _CACHE["nc"]


def _prep_w13(w):
    # [D, H] -> [HC, P, DC, P]; element [h,p,c,m] = w[c*P+p, h*P+m]
    bf = w.astype(ml_dtypes.bfloat16)
    return np.ascontiguousarray(
        bf.reshape(DC, P, HC, P).transpose(2, 1, 0, 3)
    )


def _numpy_fallback(x, w1, w2, w3, m_sizes):
    offs = np.concatenate([[0], np.cumsum(np.asarray(m_sizes, dtype=np.int64))])
    out = np.zeros((x.shape[0], w2.shape[2]), dtype=np.float32)
    for e in range(w1.shape[0]):
        xe = x[offs[e]:offs[e + 1]]
        u = xe @ w1[e]
        g = xe @ w3[e]
        z = (u / (1.0 + np.exp(-u))) * g
        out[offs[e]:offs[e + 1]] = z @ w2[e]
    return out


def kernel(x, w1, w2, w3, m_sizes, _trace=False, _trace_kwargs=None):
    global LAST_RESULTS
    x = np.ascontiguousarray(x, dtype=np.float32)
    w1 = np.ascontiguousarray(w1, dtype=np.float32)
    w2 = np.ascontiguousarray(w2, dtype=np.float32)
    w3 = np.ascontiguousarray(w3, dtype=np.float32)
    m = np.asarray(m_sizes, dtype=np.int64)

    expected = (
        x.shape == (T, D)
        and w1.shape == (E, D, H)
        and w2.shape == (E, H, D)
        and w3.shape == (E, D, H)
        and m.shape == (E,)
        and np.all(m == M)
    )
    if not expected:
        return _numpy_fallback(x, w1, w2, w3, m_sizes)

    from concourse.bass_utils import run_bass_kernel_spmd

    nc = _get_program()
    bf = ml_dtypes.bfloat16
    in_maps = []
    for e in range(E):
        xe = x[e * M:(e + 1) * M].astype(bf)
        in_maps.append({
            # [DC, P, M]; element [c,p,m] = x_e[m, c*P+p]
            "xT": np.ascontiguousarray(xe.T.reshape(DC, P, M)),
            "w1r": _prep_w13(w1[e]),
            "w3r": _prep_w13(w3[e]),
            "w2r": np.ascontiguousarray(w2[e].astype(bf).reshape(HC, P, D)),
        })

    res = run_bass_kernel_spmd(
        nc, in_maps, core_ids=list(range(E)),
        trace=_trace, **(_trace_kwargs or {}),
    )
    LAST_RESULTS = res
    return np.concatenate(
        [r["out"].astype(np.float32) for r in res.results], axis=0
    )
